# revision 9
# baseline (speedup 1.0000x reference)
"""MoE (dense routing) Trainium2 kernel v2: 8-core data-parallel over tokens.

Problem: nn_MixtureOfExperts_33011118637071
  N=16384 tokens, D=256 model dim, E=8 experts, H=128 gate hidden.
  gate   = softmax(relu(x @ Wg1 + bg1) @ Wg2 + bg2)          [N, E]
  h_e    = relu(x @ W1[e] + b1[e])                           [N, D]
  y      = sum_e gate[:, e] * (h_e @ W2[e] + b2[e])          [N, D]

v2 vs v1 (harness single-shot 76979 ns / marginal ~59.7 us):
  - all matmul operands bf16 (same PE rate as float32r at T=512, half the
    SBUF/DMA traffic, 2x DVE rate; end-to-end rel err ~4e-3)
  - gates normalized (x 1/sum) right after softmax-exp at [8, T] width, so
    the expert combine needs no final normalize: b2-bias PSUM-init matmul
    + W2 accumulation produce the final output directly (PSUM -> bf16 SBUF
    copy -> DMA out).
  - gate-row broadcasts move off the PE: normalized gate rows round-trip
    through DRAM and come back with stride-0 broadcast DMAs as [128, E, T]
    (compute engines cannot read partition-stride-0 APs; DMA from DRAM
    can). Split across the SP and Act HWDGE queues (one queue only
    sustains ~155 GB/s on this pattern). PE loses the 36 one-hot/broadcast
    matmuls: ~162k -> ~143k cycles/rep.
  - software pipelining for the in-order PE queue: gate matmuls batch by
    stage (pg1 x4 / pg2 x4 / sum x4) and experts split into W1 blocks (no
    gate dependency) and W2 blocks scheduled one tile behind, so the
    PE->Act->PE gate chain latency and the ~8 us egb broadcast latency
    hide behind W1 matmuls instead of stalling the PE (the serialized gate
    phase cost v1 ~10 us of PE idle per shot).
"""
import numpy as np
import ml_dtypes

import bass_rust
import concourse.bass as bass
import concourse.mybir as mybir
import concourse.tile as tile
from concourse.bass_utils import run_bass_kernel_spmd

F32 = mybir.dt.float32
BF16 = mybir.dt.bfloat16
AF = mybir.ActivationFunctionType

N, D, E, H = 16384, 256, 8, 128
NCORES = 8
TPC = N // NCORES          # tokens per core
T = 512                    # token tile (max moving free dim)
NT = TPC // T              # token tiles per core
KC = D // 128              # 128-row chunks of the model dim
GSLOTS = 4 * NT            # gate-spill rotation depth (cross-rep safety)

_CTR = [0]


def _split_multi_waits(nc, max_waits=1):
    """This container's walrus rejects >1 sync-wait per instruction; hoist
    extras onto fresh same-engine NoOps placed just before the waiter."""
    for fn in nc.m.functions:
        for bb in fn.blocks:
            out = []
            for inst in bb.instructions:
                si = inst.sync_info
                waits = list(si.on_wait) if si is not None and si.on_wait else []
                if len(waits) > max_waits:
                    for w in waits[:-max_waits]:
                        _CTR[0] += 1
                        nop = bass_rust.InstNoOp(
                            name=f"I-waitfix-{_CTR[0]}", ins=[], outs=[])
                        nop.engine = inst.engine
                        nop.sync_info = mybir.SyncInfo(on_wait=[w], on_update=[])
                        nc.register_instruction(nop)
                        out.append(nop)
                    si.on_wait = waits[-max_waits:]
                out.append(inst)
            bb.instructions = out


def build_nc(repeat: int = 1):
    nc = bass.Bass("TRN2", target_bir_lowering=False, debug=False,
                   num_devices=NCORES)

    xT_d = nc.dram_tensor("xT", [128, KC, TPC], BF16, kind="ExternalInput")
    Wg1_d = nc.dram_tensor("Wg1", [128, KC, H], BF16, kind="ExternalInput")
    bg1_d = nc.dram_tensor("bg1", [H], F32, kind="ExternalInput")
    Wg2_d = nc.dram_tensor("Wg2", [H, E], BF16, kind="ExternalInput")
    bg2_d = nc.dram_tensor("bg2", [E], F32, kind="ExternalInput")
    W1_d = nc.dram_tensor("W1", [128, E, KC, D], BF16, kind="ExternalInput")
    b1_d = nc.dram_tensor("b1", [128, E, KC], F32, kind="ExternalInput")
    W2_d = nc.dram_tensor("W2", [128, E, KC, D], BF16, kind="ExternalInput")
    b2_d = nc.dram_tensor("b2", [E, D], BF16, kind="ExternalInput")
    consts_d = nc.dram_tensor("consts", [E, E], BF16, kind="ExternalInput")
    gsp_d = nc.dram_tensor("gspill", [GSLOTS, E, T], BF16, kind="Internal")
    yT_d = nc.dram_tensor("yT", [D, TPC], BF16, kind="ExternalOutput")

    with tile.TileContext(nc) as tc:
        with (
            nc.allow_low_precision(reason="bf16 matmul operands"),
            tc.tile_pool(name="wpool", bufs=1) as wp,
            tc.tile_pool(name="work", bufs=NT + 1) as sb,
            tc.tile_pool(name="gbuf", bufs=NT + 1) as gb,
            tc.tile_pool(name="egbuf", bufs=NT + 2) as egp,
            tc.tile_pool(name="hbuf", bufs=2 * E + 2) as hb,
            tc.tile_pool(name="obuf", bufs=4) as ob,
            tc.tile_pool(name="xpool", bufs=2) as xp,
            tc.tile_pool(name="pgate", bufs=2, space="PSUM") as pgate,
            tc.tile_pool(name="phid", bufs=3, space="PSUM") as phid,
            tc.tile_pool(name="pout", bufs=3, space="PSUM") as pout,
        ):
            # head: gate weights + x tile 0 on SP; the other x tiles + gate
            # biases on Act; expert weights stream in behind on both HWDGE
            # queues (emitted after the gate ops so the tiny gate-path DMAs
            # aren't stuck behind 4 MB of weights).
            wg1 = wp.tile([128, KC, H], BF16, tag="wg1")
            nc.sync.dma_start(wg1[:, :, :], Wg1_d[:, :, :])
            wg2 = wp.tile([H, E], BF16, tag="wg2")
            bg1 = wp.tile([H, 1], F32, tag="bg1")
            bg2 = wp.tile([E, 1], F32, tag="bg2")
            on8x8 = wp.tile([E, E], BF16, tag="on8x8")

            def load_gate_smalls():
                nc.sync.dma_start(wg2[:, :], Wg2_d[:, :])
                nc.scalar.dma_start(bg1[:, 0], bg1_d[:])
                nc.scalar.dma_start(bg2[:, 0], bg2_d[:])
                nc.scalar.dma_start(on8x8[:, :], consts_d[:, :])

            w1 = wp.tile([128, E, KC, D], BF16, tag="w1")
            w2 = wp.tile([128, E, KC, D], BF16, tag="w2")
            b1t = wp.tile([128, E, KC], F32, tag="b1t")
            b2t = wp.tile([E, D], BF16, tag="b2t")

            def load_w1():
                nc.scalar.dma_start(b1t[:, :, :], b1_d[:, :, :])
                # halves are contiguous: one big DMA per queue
                nc.sync.dma_start(w1[:, 0:E // 2, :, :],
                                  W1_d[:, 0:E // 2, :, :])
                nc.scalar.dma_start(w1[:, E // 2:E, :, :],
                                    W1_d[:, E // 2:E, :, :])

            def load_w2():
                nc.sync.dma_start(b2t[:, :], b2_d[:, :])
                nc.sync.dma_start(w2[:, 0:E // 2, :, :],
                                  W2_d[:, 0:E // 2, :, :])
                nc.scalar.dma_start(w2[:, E // 2:E, :, :],
                                    W2_d[:, E // 2:E, :, :])

            def load_x(xt, rep, first=False):
                for ti in range(NT):
                    tok = slice(ti * T, (ti + 1) * T)
                    # rep 0: split across queues for the fastest head; later
                    # reps: keep the Act sequencer free of DMA issues (it is
                    # the busiest engine queue in steady state)
                    q = (nc.sync if ti == 0 or ti == 2 else nc.scalar) \
                        if first else nc.sync
                    q.dma_start(xt[:, :, tok], xT_d[:, :, tok])

            def gate_pg1(xt, ti, rep):
                tok = slice(ti * T, (ti + 1) * T)
                pg1 = pgate.tile([128, T], F32, tag="pg", name=f"pg1_{rep}_{ti}")
                for kc in range(KC):
                    nc.tensor.matmul(pg1[:, :], wg1[:, kc, :], xt[:, kc, tok],
                                     start=(kc == 0), stop=(kc == KC - 1))
                rh = sb.tile([H, T], BF16, tag="rh", name=f"rh_{rep}_{ti}")
                nc.scalar.activation(rh[:, :], pg1[:, :], AF.Relu,
                                     bias=bg1[:, 0:1])
                return rh

            def gate_pg2(rh, ti, rep):
                pg2 = pgate.tile([E, T], F32, tag="pg", name=f"pg2_{rep}_{ti}")
                nc.tensor.matmul(pg2[:, :], wg2[:, :], rh[:, :],
                                 start=True, stop=True)
                expl = gb.tile([E, T], BF16, tag="expl", name=f"expl_{rep}_{ti}")
                nc.scalar.activation(expl[:, :], pg2[:, :], AF.Exp,
                                     bias=bg2[:, 0:1])
                return expl

            def gate_sum(expl, ti, rep):
                slot = (rep * NT + ti) % GSLOTS
                # all-ones [8,8] stationary: sums over experts AND broadcasts
                # the sum to all 8 partitions in one matmul (same 512 cycles
                # as a plain [8,1] sum)
                psum = pgate.tile([E, T], F32, tag="pg", name=f"ps_{rep}_{ti}")
                nc.tensor.matmul(psum[:, :], on8x8[:, :], expl[:, :],
                                 start=True, stop=True)
                rec8 = gb.tile([E, T], BF16, tag="rec8", name=f"rec8_{rep}_{ti}")
                nc.vector.reciprocal(rec8[:, :], psum[:, :])
                gn = gb.tile([E, T], BF16, tag="gn", name=f"gn_{rep}_{ti}")
                nc.vector.tensor_mul(gn[:, :], expl[:, :], rec8[:, :])
                return gn, slot

            def emit_gspill(slot, gn, q=None):
                (q or nc.sync).dma_start(gsp_d[slot, :, :], gn[:, :])

            def emit_egb(ti, rep, slot, q1=None, q2=None):
                if q2 is None and rep > 0:
                    q2 = nc.gpsimd  # keep the Act queue free of DMA issues
                # normalized gate rows -> DRAM -> stride-0 broadcast back as
                # [128, E, T] (each partition gets all 8 rows), split across
                # both HWDGE queues; low experts on SP (needed first).
                egb = egp.tile([128, E, T], BF16, tag="egb",
                               name=f"egb_{rep}_{ti}")
                half = E // 2
                (q1 or nc.sync).dma_start(
                    egb[:, 0:half, :],
                    gsp_d[slot, 0:half, :].unsqueeze(0)
                    .to_broadcast([128, half, T]))
                (q2 or nc.scalar).dma_start(
                    egb[:, half:E, :],
                    gsp_d[slot, half:E, :].unsqueeze(0)
                    .to_broadcast([128, half, T]))
                return egb

            def experts_w1(xt, ti, rep, egb):
                """W1 GEMMs + relu + gate multiply for all experts of tile ti.
                Only the DVE multiply depends on the gate (egb)."""
                tok = slice(ti * T, (ti + 1) * T)
                hss = []
                for e in range(E):
                    hs = hb.tile([128, KC, T], BF16, tag="hs",
                                 name=f"hs_{rep}_{ti}_{e}")
                    for mc in range(KC):
                        ph = phid.tile([128, T], F32, tag="ph",
                                       name=f"ph_{rep}_{ti}_{e}_{mc}")
                        for kc in range(KC):
                            nc.tensor.matmul(
                                ph[:, :], w1[:, e, kc, mc * 128:(mc + 1) * 128],
                                xt[:, kc, tok],
                                start=(kc == 0), stop=(kc == KC - 1))
                        nc.scalar.activation(hs[:, mc, :], ph[:, :], AF.Relu,
                                             bias=b1t[:, e, mc:mc + 1])
                    # one gate multiply for both mc chunks (egb row
                    # free-dim-broadcast across the mc axis)
                    nc.vector.tensor_mul(
                        hs[:, :, :], hs[:, :, :],
                        egb[:, e, :].unsqueeze(1).to_broadcast([128, KC, T]))
                    hss.append(hs)
                return hss

            def experts_w2(ti, rep, gn, hss, last=False):
                tok = slice(ti * T, (ti + 1) * T)
                py = [pout.tile([128, T], F32, tag="py", name=f"py{mc}_{rep}_{ti}")
                      for mc in range(KC)]
                for mc in range(KC):
                    nc.tensor.matmul(py[mc][:, :],
                                     b2t[:, mc * 128:(mc + 1) * 128],
                                     gn[:, :], start=True, stop=False)
                for e in range(E):
                    for mc in range(KC):
                        for kc in range(KC):
                            nc.tensor.matmul(
                                py[mc][:, :],
                                w2[:, e, kc, mc * 128:(mc + 1) * 128],
                                hss[e][:, kc, :],
                                start=False,
                                stop=(e == E - 1 and kc == KC - 1))
                for mc in range(KC):
                    ot = ob.tile([128, T], BF16, tag="ot",
                                 name=f"ot_{rep}_{ti}_{mc}")
                    nc.vector.tensor_copy(ot[:, :], py[mc][:, :])
                    # very last store rides the by-then-idle HWDGE queues
                    # (SWDGE adds ~1us of Q7 descriptor generation)
                    q = (nc.sync if mc == 0 else nc.scalar) if last \
                        else nc.gpsimd
                    q.dma_start(yT_d[mc * 128:(mc + 1) * 128, tok], ot[:, :])

            def emit_gates(xt, rep):
                """Batched by stage: the PE runs pg1 x4 back-to-back, then
                pg2 x4 (each tile's relu finished during the pg1 batch),
                then sum x4. No PE stalls on the Act round trips.
                At rep 0, tile 0's chain runs start-to-finish first so its
                egb broadcast (the critical W2(t0) input) launches ~2.5us
                earlier; the head PE is waiting on x loads then anyway."""
                if rep == 0:
                    rh0 = gate_pg1(xt, 0, rep)
                    ex0 = gate_pg2(rh0, 0, rep)
                    g0 = gate_sum(ex0, 0, rep)
                    rhs = [gate_pg1(xt, ti, rep) for ti in range(1, NT)]
                    expls = [gate_pg2(rhs[i], i + 1, rep) for i in range(NT - 1)]
                    return [g0] + [gate_sum(expls[i], i + 1, rep)
                                   for i in range(NT - 1)]
                rhs = [gate_pg1(xt, ti, rep) for ti in range(NT)]
                expls = [gate_pg2(rhs[ti], ti, rep) for ti in range(NT)]
                return [gate_sum(expls[ti], ti, rep) for ti in range(NT)]

            xts, gates, egbs = {}, {}, {}
            for rep in range(repeat):
                if rep == 0:
                    xts[0] = xp.tile([128, KC, TPC], BF16, tag="xt", name="xt0")
                    load_x(xts[0], 0, first=True)
                    load_gate_smalls()
                    gates[0] = emit_gates(xts[0], 0)
                    # weights first (ready immediately; the gate spills wait
                    # on compute and would head-block the queue), with the
                    # egb chains interleaved between the w1 and w2 batches
                    load_w1()
                    # tile 0's gate broadcast takes the idle SWDGE queue so
                    # it isn't stuck behind 2 MB of W1 on the HWDGE queues
                    emit_gspill(gates[0][0][1], gates[0][0][0], q=nc.gpsimd)
                    egbs[(0, 0)] = emit_egb(0, 0, gates[0][0][1],
                                            q1=nc.gpsimd, q2=nc.gpsimd)
                    load_w2()
                    for ti in range(1, NT):
                        emit_gspill(gates[0][ti][1], gates[0][ti][0])
                        egbs[(0, ti)] = emit_egb(ti, 0, gates[0][ti][1])
                xt, gs = xts[rep], gates[rep]
                # W1 blocks run one tile ahead of W2 blocks so the egb
                # broadcast latency and the PSUM-bank recycling hide behind
                # W1 matmuls; next rep's x-load + gates slot in before the
                # last W2 block.
                hss = {}
                hss[0] = experts_w1(xt, 0, rep, egbs.pop((rep, 0)))
                hss[1] = experts_w1(xt, 1, rep, egbs.pop((rep, 1)))
                experts_w2(0, rep, gs[0][0], hss.pop(0))
                hss[2] = experts_w1(xt, 2, rep, egbs.pop((rep, 2)))
                if rep + 1 < repeat:
                    xts[rep + 1] = xp.tile([128, KC, TPC], BF16, tag="xt",
                                           name=f"xt{rep + 1}")
                    load_x(xts[rep + 1], rep + 1)
                experts_w2(1, rep, gs[1][0], hss.pop(1))
                hss[3] = experts_w1(xt, 3, rep, egbs.pop((rep, 3)))
                if rep + 1 < repeat:
                    gates[rep + 1] = emit_gates(xts[rep + 1], rep + 1)
                    for ti in range(NT):
                        emit_gspill(gates[rep + 1][ti][1],
                                    gates[rep + 1][ti][0])
                    for ti in range(NT):
                        egbs[(rep + 1, ti)] = emit_egb(
                            ti, rep + 1, gates[rep + 1][ti][1])
                    del xts[rep], gates[rep]
                experts_w2(2, rep, gs[2][0], hss.pop(2))
                experts_w2(3, rep, gs[3][0], hss.pop(3),
                           last=(rep == repeat - 1))

    _split_multi_waits(nc)
    return nc


_NC_CACHE = None


def _get_nc():
    global _NC_CACHE
    if _NC_CACHE is None:
        _NC_CACHE = build_nc()
    return _NC_CACHE


def _bf(a):
    return np.ascontiguousarray(np.asarray(a, np.float32)).astype(
        ml_dtypes.bfloat16)


def _dev_w(W):
    """[E, D, D] -> [128, E, KC, D]: partition-major device layout so the
    weight DMA is one contiguous run per partition."""
    W = np.asarray(W, np.float32).reshape(E, KC, 128, D)
    return np.ascontiguousarray(W.transpose(2, 0, 1, 3))


def make_in_maps(x, Wg1, bg1, Wg2, bg2, W1, b1, W2, b2):
    x = np.ascontiguousarray(np.asarray(x, dtype=np.float32))
    xT = x.T.reshape(KC, 128, N)              # [KC, 128, N]
    xTb = _bf(np.ascontiguousarray(xT.transpose(1, 0, 2)))  # [128, KC, N]
    Wg1r = np.asarray(Wg1, np.float32).reshape(KC, 128, H).transpose(1, 0, 2)
    b1r = np.asarray(b1, np.float32).reshape(E, KC, 128).transpose(2, 0, 1)
    shared = {
        "Wg1": _bf(Wg1r),
        "bg1": np.ascontiguousarray(np.asarray(bg1, np.float32)),
        "Wg2": _bf(Wg2),
        "bg2": np.ascontiguousarray(np.asarray(bg2, np.float32)),
        "W1": _bf(_dev_w(W1)),
        "b1": np.ascontiguousarray(b1r),
        "W2": _bf(_dev_w(W2)),
        "b2": _bf(b2),
        "consts": np.ones((E, E), ml_dtypes.bfloat16),
    }
    return [
        {"xT": np.ascontiguousarray(xTb[:, :, c * TPC:(c + 1) * TPC]),
         **shared}
        for c in range(NCORES)
    ]


def gather_output(results):
    out = np.empty((N, D), np.float32)
    for c in range(NCORES):
        out[c * TPC:(c + 1) * TPC, :] = results[c]["yT"].T.astype(np.float32)
    return out


def kernel(x, Wg1, bg1, Wg2, bg2, W1, b1, W2, b2):
    nc = _get_nc()
    in_maps = make_in_maps(x, Wg1, bg1, Wg2, bg2, W1, b1, W2, b2)
    r = run_bass_kernel_spmd(nc, in_maps, list(range(NCORES)))
    return gather_output(r.results)
